# revision 32
# baseline (speedup 1.0000x reference)
"""Trainium2 Bass kernel for multi-relation SpMM (gnn message passing).

out = concat([A_0 @ x, A_1 @ x, A_2 @ x, x], axis=1)  where A_r is a sparse
COO adjacency given by (edge_rows[r], edge_cols[r], edge_vals[r]).

Sharding: destination rows split across 8 cores (6250 rows each). Each core
gathers source features per edge with SWDGE dma_gather from a bf16 pair table
(row i holds x[2i]|x[2i+1], 256B per descriptor; idx = col >> 1), builds a
val-scaled one-hot scatter matrix per 128-edge chunk, and accumulates chunk
matmuls (bf16) into PSUM per 128-row output block on PE.

Perf-critical detail: DVE 2-port perf-mode ops take an exclusive lock on the
SBUF port pair shared with GPSIMD, starving SWDGE descriptor generation. The
one-hot builds therefore avoid 2-port modes entirely: the DVE route reads the
iota from PSUM (PSUM has one DVE read port, forcing 1x single-port mode) and
the ACT route (own ports) builds the one-hot as val*relu(1 - (iota-lrow)^2).
"""

import sys

sys.path.insert(0, "/opt/trn_rl_repo")

# antenv.axon_hooks is missing from the staged repo; provide it so the axon
# trn boot can register the NTFF profile hook (enables trace/exec-time).
try:
    import antenv.axon_hooks  # noqa: F401
except ImportError:
    import types

    import antenv

    _m = types.ModuleType("antenv.axon_hooks")
    _m._hook = None

    def _set_hook(h, _m=_m):
        _m._hook = h

    def _get_hook(_m=_m):
        return _m._hook

    _m.set_axon_ntff_profile_hook = _set_hook
    _m.get_axon_ntff_profile_hook = _get_hook
    sys.modules["antenv.axon_hooks"] = _m
    antenv.axon_hooks = _m

    # boot() ran at interpreter start (sitecustomize) before this module
    # existed, so its hook registration was silently skipped. Redo it.
    try:
        from trn_agent_boot.trn_boot import _ntff_profile_via_ctypes

        _set_hook(_ntff_profile_via_ctypes("/opt/axon/libaxon_pjrt.so"))
    except Exception:
        pass

from contextlib import ExitStack

import ml_dtypes
import numpy as np

import concourse.bacc as bacc
import concourse.tile as tile
from concourse import mybir
from concourse.bass_utils import run_bass_kernel_spmd

P = 128
BF16 = ml_dtypes.bfloat16


class Config:
    def __init__(self, N, D, R, ncores=8, bg=4, act_every=3):
        assert N % (2 * ncores) == 0
        self.N, self.D, self.R, self.ncores = N, D, R, ncores
        self.NPC = N // ncores                     # rows per core
        self.NB = (self.NPC + P - 1) // P          # 128-row blocks per core
        self.NBP = self.NB * P                     # padded rows per core
        self.BG = bg                               # blocks per group
        self.NG = (self.NB + bg - 1) // bg         # groups
        self.NSRC = N // 2                         # rows in the pair table
        self.RD1 = (R + 1) * D
        self.ACT_EVERY = act_every                 # 1/act_every chunks on ACT


def _schedule(cfg, edge_rows, edge_cols):
    """Per-(relation, parity, block) chunk counts, shared across cores."""
    R, NB, NPC, ncores = cfg.R, cfg.NB, cfg.NPC, cfg.ncores
    cnt = np.zeros((ncores, R, 2, NB), dtype=np.int64)
    for r in range(R):
        er = np.asarray(edge_rows[r])
        ec = np.asarray(edge_cols[r])
        core = er // NPC
        b = (er % NPC) // P
        par = ec & 1
        flat = ((core * R + r) * 2 + par) * NB + b
        c = np.bincount(flat.ravel(), minlength=ncores * R * 2 * NB)
        cnt += c.reshape(ncores, R, 2, NB)
    nch = (cnt.max(axis=0) + P - 1) // P          # [R, 2, NB]
    # ensure every block has at least one chunk so PSUM gets initialized
    empty = nch.sum(axis=1) == 0                  # [R, NB]
    for r in range(R):
        nch[r, 0, empty[r]] = 1
    return nch.astype(np.int64)


def _layout(cfg, nch):
    """Global chunk offsets in program order: (group, relation, parity, block).

    One gather call per (group, relation) covering both parities (the pair
    table is parity-agnostic; parity only selects the rhs half per chunk).
    """
    NB, BG, NG, R = cfg.NB, cfg.BG, cfg.NG, cfg.R
    block_off = np.zeros((R, 2, NB), dtype=np.int64)
    calls = []  # (g, r, chunk_off, n_chunks)
    off = 0
    for g in range(NG):
        bs = range(g * BG, min((g + 1) * BG, NB))
        for r in range(R):
            cc = 0
            for par in (0, 1):
                for b in bs:
                    block_off[r, par, b] = off + cc
                    cc += int(nch[r, par, b])
            calls.append((g, r, off, cc))
            off += cc
    return block_off, calls, off  # off == total chunks CT


def _prepare_core(cfg, core, nch, block_off, CT, edge_rows, edge_cols, edge_vals):
    """Build this core's linear edge arrays (idx, lrow, val) of length CT*128."""
    R, NPC = cfg.R, cfg.NPC
    lin_idx = np.zeros(CT * P, dtype=np.int16)
    lin_lrow = np.zeros(CT * P, dtype=np.float32)
    lin_val = np.zeros(CT * P, dtype=np.float32)
    for r in range(R):
        er = np.asarray(edge_rows[r])
        m = (er // NPC) == core
        rows = er[m] % NPC
        cols = np.asarray(edge_cols[r])[m]
        vals = np.asarray(edge_vals[r])[m]
        b = rows // P
        lrow = rows % P
        par = cols & 1
        idx = cols >> 1
        grp = par.astype(np.int64) * cfg.NB + b
        order = np.argsort(grp, kind="stable")
        gs = grp[order]
        # rank within each (par, b) bucket
        starts = np.r_[0, np.flatnonzero(np.diff(gs)) + 1]
        sizes = np.diff(np.r_[starts, len(gs)])
        rank = np.arange(len(gs)) - np.repeat(starts, sizes)
        pg, bg_ = gs // cfg.NB, gs % cfg.NB
        pos = block_off[r, pg, bg_] * P + rank
        lin_idx[pos] = idx[order].astype(np.int16)
        lin_lrow[pos] = lrow[order].astype(np.float32)
        lin_val[pos] = vals[order].astype(np.float32)
    return lin_idx, lin_lrow, lin_val


def _wrap_idx(cfg, calls, CT, lin_idx):
    """SWDGE index layout: within each call, edge i -> [i%16, i//16], x8 groups.

    A chunk-aligned sub-range of a call is itself a valid wrap (the column
    offset is 8 per chunk), so gather calls may be split device-side without
    re-wrapping.
    """
    idx_arr = np.zeros((P, CT * 8), dtype=np.int16)
    for (_, _, off, cc) in calls:
        if cc == 0:
            continue
        w = lin_idx[off * P : (off + cc) * P].reshape(cc * 8, 16).T  # [16, cc*8]
        idx_arr[:, off * 8 : (off + cc) * 8] = np.tile(w, (8, 1))
    return idx_arr


def _tag(cg):
    """Per-chunk one-hot build route: 'M' host-DMA, 'V' DVE, 'A' ACT."""
    t = cg % 30
    if t < 20:
        return "M"
    if t < 27:
        return "V"
    return "A"


def _dma_chunks(calls):
    """Chunks whose scatter matrix is host-built, in call-major order."""
    out = []
    for (_, _, off, cc) in calls:
        out.extend(cg for cg in range(off, off + cc) if _tag(cg) == "M")
    return out


def _build(cfg, nch, block_off, calls, CT, skip=()):
    f32 = mybir.dt.float32
    bf16 = mybir.dt.bfloat16
    nc = bacc.Bacc(
        "TRN2",
        target_bir_lowering=False,
        debug=False,
        num_devices=cfg.ncores,
        num_swdge_queues=4,
    )
    D, R, BG, NG, NB, RD1 = cfg.D, cfg.R, cfg.BG, cfg.NG, cfg.NB, cfg.RD1
    xpair = nc.dram_tensor("xpair", [cfg.NSRC, 2 * D], bf16, kind="ExternalInput").ap()
    x_own = nc.dram_tensor("x_own", [NG, P, BG, D], f32, kind="ExternalInput").ap()
    idx_d = nc.dram_tensor("idx", [P, CT * 8], mybir.dt.int16, kind="ExternalInput").ap()
    sc_d = nc.dram_tensor("sc", [P, 4, CT], f32, kind="ExternalInput").ap()
    iota_d = nc.dram_tensor("iota", [P, P], f32, kind="ExternalInput").ap()
    out_d = nc.dram_tensor("out", [cfg.NBP, RD1], f32, kind="ExternalOutput").ap()
    dma_cgs = _dma_chunks(calls)
    TOTM = max(1, len(dma_cgs))
    mv_d = nc.dram_tensor("mvs", [P, TOTM, P], bf16, kind="ExternalInput").ap()
    mslot = {cg: j for j, cg in enumerate(dma_cgs)}

    calls_by_g = {}
    for (g, r, off, cc) in calls:
        calls_by_g.setdefault(g, {})[r] = (off, cc)

    with tile.TileContext(nc) as tc, ExitStack() as ctx:
        cpool = ctx.enter_context(tc.tile_pool(name="c", bufs=1))
        gpool = ctx.enter_context(tc.tile_pool(name="g", bufs=26))
        hpool = ctx.enter_context(tc.tile_pool(name="h", bufs=3))
        ipool = ctx.enter_context(tc.tile_pool(name="i", bufs=3))
        kpool = ctx.enter_context(tc.tile_pool(name="k", bufs=3))
        mpool = ctx.enter_context(tc.tile_pool(name="m", bufs=8))
        opool = ctx.enter_context(tc.tile_pool(name="o", bufs=2))
        ppool = ctx.enter_context(tc.tile_pool(name="p", bufs=4, space="PSUM"))
        qpool = ctx.enter_context(tc.tile_pool(name="q", bufs=1, space="PSUM"))
        spool = ctx.enter_context(tc.tile_pool(name="s", bufs=2, space="PSUM"))

        iota_t = cpool.tile([P, P], f32)
        nc.sync.dma_start(out=iota_t[:], in_=iota_d[:])

        # iota in PSUM: forces DVE tensor_scalar into 1x single-port mode so
        # it never locks the SBUF port pair GPSIMD needs for SWDGE descriptors.
        iota_ps = qpool.tile([P, P], f32, space="PSUM")
        nc.scalar.copy(iota_ps[:], iota_t[:])

        nsub_q = 0
        for g in range(NG):
            bs = list(range(g * BG, min((g + 1) * BG, NB)))
            goff = calls_by_g[g][0][0]
            gend = calls_by_g[g][R - 1][0] + calls_by_g[g][R - 1][1]
            ccg = gend - goff
            # stream this group's gather indices and one-hot scalars
            idx_t = ipool.tile([P, ccg * 8], mybir.dt.int16)
            nc.sync.dma_start(out=idx_t[:], in_=idx_d[:, goff * 8 : gend * 8])
            sct = kpool.tile([P, 4, ccg], f32)
            nc.sync.dma_start(out=sct[:], in_=sc_d[:, :, goff:gend])
            ot = opool.tile([P, BG, RD1], f32)
            if "identity" not in skip:
                # scalar (ACT) HWDGE queue: keeps the sync queue free for the
                # gather-critical idx/sc loads.
                nc.scalar.dma_start(out=ot[:, :, R * D :], in_=x_own[g])
            for r in range(R):
                off, cc = calls_by_g[g][r]
                if cc == 0 or "gather" in skip:
                    continue
                # Split each call into 4 sub-gathers cycling the 4 SWDGE
                # queues: each queue's descriptors are generated by a
                # different Q7 core pair, so descriptor generation (the
                # Q7-software bottleneck) pipelines across pairs.
                subs = []
                nsub = min(4, cc)
                base, rem = divmod(cc, nsub)
                sizes = [base + (1 if i < rem else 0) for i in range(nsub)]
                nsub_q += 1  # stagger: rotate which queue starts each call
                soff = off
                for scc in sizes:
                    if scc == 0:
                        continue
                    t = gpool.tile([P, scc, 2 * D], bf16)
                    nc.gpsimd.dma_gather(
                        out_ap=t[:],
                        in_ap=xpair[:],
                        idxs_ap=idx_t[:, (soff - goff) * 8 : (soff - goff + scc) * 8],
                        num_idxs=scc * P,
                        num_idxs_reg=scc * P,
                        elem_size=2 * D,
                        single_packet=False,
                        queue_num=nsub_q % 4,
                    )
                    nsub_q += 1
                    subs.append((soff, scc, t))
                    soff += scc
                # host-built scatter matrices for this call's 'M' chunks
                mtiles = [cg for cg in range(off, off + cc) if _tag(cg) == "M"]
                if mtiles and "mvdma" not in skip:
                    mvs = hpool.tile([P, len(mtiles), P], bf16)
                    j0 = mslot[mtiles[0]]
                    nc.scalar.dma_start(
                        out=mvs[:], in_=mv_d[:, j0 : j0 + len(mtiles), :]
                    )
                for b4, b in enumerate(bs):
                    if "mm" in skip:
                        continue
                    total = int(nch[r, 0, b] + nch[r, 1, b])
                    acc = ppool.tile([P, D], f32, space="PSUM")
                    k = 0
                    for par in (0, 1):
                        n = int(nch[r, par, b])
                        if n == 0:
                            continue
                        boff = int(block_off[r, par, b])
                        for ci in range(n):
                            cg = boff + ci
                            cgl = cg - goff
                            si = 0
                            while si + 1 < len(subs) and cg >= subs[si + 1][0]:
                                si += 1
                            sub = subs[si]
                            t = sub[2]
                            cl = cg - sub[0]
                            tag = _tag(cg)
                            if tag == "M" and "mvdma" not in skip:
                                mv_ap = mvs[:, mslot[cg] - j0, :]
                            elif tag == "A" and "act" not in skip:
                                # ACT route: val * relu(1 - (iota - lrow)^2)
                                mv = mpool.tile([P, P], bf16)
                                sq = spool.tile([P, P], f32, space="PSUM")
                                nc.scalar.activation(
                                    sq[:],
                                    iota_ps[:],
                                    mybir.ActivationFunctionType.Square,
                                    bias=sct[:, 2, cgl : cgl + 1],
                                )
                                nc.scalar.activation(
                                    mv[:],
                                    sq[:],
                                    mybir.ActivationFunctionType.Relu,
                                    bias=sct[:, 1, cgl : cgl + 1],
                                    scale=sct[:, 3, cgl : cgl + 1],
                                )
                                mv_ap = mv[:]
                            else:
                                mv = mpool.tile([P, P], bf16)
                                nc.vector.tensor_scalar(
                                    out=mv[:],
                                    in0=iota_ps[:],
                                    scalar1=sct[:, 0, cgl : cgl + 1],
                                    scalar2=sct[:, 1, cgl : cgl + 1],
                                    op0=mybir.AluOpType.is_equal,
                                    op1=mybir.AluOpType.mult,
                                )
                                mv_ap = mv[:]
                            if "matmul" not in skip:
                                nc.tensor.matmul(
                                    out=acc[:],
                                    lhsT=mv_ap,
                                    rhs=t[:, cl, par * D : (par + 1) * D],
                                    start=(k == 0),
                                    stop=(k == total - 1),
                                )
                            k += 1
                    if "matmul" not in skip and "copy" not in skip:
                        nc.scalar.copy(ot[:, b4, r * D : (r + 1) * D], acc[:])
            for b4, b in enumerate(bs):
                if "out" in skip:
                    continue
                nc.scalar.dma_start(
                    out=out_d[b * P : (b + 1) * P, :], in_=ot[:, b4, :]
                )
    nc.compile()
    return nc


_CACHE = {}


def _get_kernel(cfg, nch, block_off, calls, CT):
    key = (cfg.N, cfg.D, cfg.R, cfg.ncores, cfg.ACT_EVERY, nch.tobytes())
    if key not in _CACHE:
        _CACHE[key] = _build(cfg, nch, block_off, calls, CT)
    return _CACHE[key]


def run(x, edge_rows, edge_cols, edge_vals, cfg=None, trace=False, tmpdir=None):
    x = np.ascontiguousarray(np.asarray(x, dtype=np.float32))
    edge_rows = np.asarray(edge_rows, dtype=np.int64)
    edge_cols = np.asarray(edge_cols, dtype=np.int64)
    edge_vals = np.asarray(edge_vals, dtype=np.float32)
    if cfg is None:
        cfg = Config(x.shape[0], x.shape[1], edge_rows.shape[0])

    nch = _schedule(cfg, edge_rows, edge_cols)
    block_off, calls, CT = _layout(cfg, nch)
    nc = _get_kernel(cfg, nch, block_off, calls, CT)

    iota = np.broadcast_to(np.arange(P, dtype=np.float32), (P, P))
    xpair = np.ascontiguousarray(x.reshape(cfg.NSRC, 2 * cfg.D)).astype(BF16)
    dma_cgs = np.asarray(_dma_chunks(calls), dtype=np.int64)
    TOTM = max(1, len(dma_cgs))
    in_maps = []
    for core in range(cfg.ncores):
        lin_idx, lin_lrow, lin_val = _prepare_core(
            cfg, core, nch, block_off, CT, edge_rows, edge_cols, edge_vals
        )
        idx_arr = _wrap_idx(cfg, calls, CT, lin_idx)
        lrow_arr = np.ascontiguousarray(lin_lrow.reshape(CT, P).T)
        val_arr = np.ascontiguousarray(lin_val.reshape(CT, P).T)
        sc = np.ascontiguousarray(
            np.stack([lrow_arr, val_arr, -lrow_arr, -val_arr], axis=1)
        )
        # host-built scatter matrices for the 'M'-tagged chunks
        mvs_all = np.zeros((TOTM, P, P), dtype=np.float32)
        if len(dma_cgs):
            lr = lin_lrow.reshape(CT, P)[dma_cgs].astype(np.int64)  # [TOTM, P]
            vv = lin_val.reshape(CT, P)[dma_cgs]
            mvs_all[
                np.arange(len(dma_cgs))[:, None], np.arange(P)[None, :], lr
            ] = vv
        mvs_all = np.ascontiguousarray(mvs_all.transpose(1, 0, 2)).astype(BF16)
        xpad = np.zeros((cfg.NG * cfg.BG * P, cfg.D), dtype=np.float32)
        xpad[: cfg.NPC] = x[core * cfg.NPC : (core + 1) * cfg.NPC]
        x_own = np.ascontiguousarray(
            xpad.reshape(cfg.NG, cfg.BG, P, cfg.D).transpose(0, 2, 1, 3)
        )
        in_maps.append(
            {
                "xpair": xpair,
                "x_own": x_own,
                "idx": idx_arr,
                "sc": sc,
                "iota": np.ascontiguousarray(iota),
                "mvs": mvs_all,
            }
        )

    res = run_bass_kernel_spmd(
        nc, in_maps, list(range(cfg.ncores)), trace=trace, tmpdir=tmpdir
    )
    out = np.concatenate(
        [res.results[i]["out"][: cfg.NPC] for i in range(cfg.ncores)], axis=0
    )
    return out, res


def kernel(x, edge_rows, edge_cols, edge_vals):
    out, _ = run(x, edge_rows, edge_cols, edge_vals)
    return out


# revision 33
# speedup vs baseline: 1.0082x; 1.0082x over previous
"""Trainium2 Bass kernel for multi-relation SpMM (gnn message passing).

out = concat([A_0 @ x, A_1 @ x, A_2 @ x, x], axis=1)  where A_r is a sparse
COO adjacency given by (edge_rows[r], edge_cols[r], edge_vals[r]).

Sharding: destination rows split across 8 cores (6250 rows each). Each core
gathers source features per edge with SWDGE dma_gather from a bf16 pair table
(row i holds x[2i]|x[2i+1], 256B per descriptor; idx = col >> 1), builds a
val-scaled one-hot scatter matrix per 128-edge chunk, and accumulates chunk
matmuls (bf16) into PSUM per 128-row output block on PE.

Perf-critical detail: DVE 2-port perf-mode ops take an exclusive lock on the
SBUF port pair shared with GPSIMD, starving SWDGE descriptor generation. The
one-hot builds therefore avoid 2-port modes entirely: the DVE route reads the
iota from PSUM (PSUM has one DVE read port, forcing 1x single-port mode) and
the ACT route (own ports) builds the one-hot as val*relu(1 - (iota-lrow)^2).
"""

import sys

sys.path.insert(0, "/opt/trn_rl_repo")

# antenv.axon_hooks is missing from the staged repo; provide it so the axon
# trn boot can register the NTFF profile hook (enables trace/exec-time).
try:
    import antenv.axon_hooks  # noqa: F401
except ImportError:
    import types

    import antenv

    _m = types.ModuleType("antenv.axon_hooks")
    _m._hook = None

    def _set_hook(h, _m=_m):
        _m._hook = h

    def _get_hook(_m=_m):
        return _m._hook

    _m.set_axon_ntff_profile_hook = _set_hook
    _m.get_axon_ntff_profile_hook = _get_hook
    sys.modules["antenv.axon_hooks"] = _m
    antenv.axon_hooks = _m

    # boot() ran at interpreter start (sitecustomize) before this module
    # existed, so its hook registration was silently skipped. Redo it.
    try:
        from trn_agent_boot.trn_boot import _ntff_profile_via_ctypes

        _set_hook(_ntff_profile_via_ctypes("/opt/axon/libaxon_pjrt.so"))
    except Exception:
        pass

from contextlib import ExitStack

import ml_dtypes
import numpy as np

import concourse.bacc as bacc
import concourse.tile as tile
from concourse import mybir
from concourse.bass_utils import run_bass_kernel_spmd

P = 128
BF16 = ml_dtypes.bfloat16


class Config:
    def __init__(self, N, D, R, ncores=8, bg=4, act_every=3):
        assert N % (2 * ncores) == 0
        self.N, self.D, self.R, self.ncores = N, D, R, ncores
        self.NPC = N // ncores                     # rows per core
        self.NB = (self.NPC + P - 1) // P          # 128-row blocks per core
        self.NBP = self.NB * P                     # padded rows per core
        self.BG = bg                               # blocks per group
        self.NG = (self.NB + bg - 1) // bg         # groups
        self.NSRC = N // 2                         # rows in the pair table
        self.RD1 = (R + 1) * D
        self.ACT_EVERY = act_every                 # 1/act_every chunks on ACT


def _schedule(cfg, edge_rows, edge_cols):
    """Per-(relation, parity, block) chunk counts, shared across cores."""
    R, NB, NPC, ncores = cfg.R, cfg.NB, cfg.NPC, cfg.ncores
    cnt = np.zeros((ncores, R, 2, NB), dtype=np.int64)
    for r in range(R):
        er = np.asarray(edge_rows[r])
        ec = np.asarray(edge_cols[r])
        core = er // NPC
        b = (er % NPC) // P
        par = ec & 1
        flat = ((core * R + r) * 2 + par) * NB + b
        c = np.bincount(flat.ravel(), minlength=ncores * R * 2 * NB)
        cnt += c.reshape(ncores, R, 2, NB)
    nch = (cnt.max(axis=0) + P - 1) // P          # [R, 2, NB]
    # ensure every block has at least one chunk so PSUM gets initialized
    empty = nch.sum(axis=1) == 0                  # [R, NB]
    for r in range(R):
        nch[r, 0, empty[r]] = 1
    return nch.astype(np.int64)


def _layout(cfg, nch):
    """Global chunk offsets in program order: (group, relation, parity, block).

    One gather call per (group, relation) covering both parities (the pair
    table is parity-agnostic; parity only selects the rhs half per chunk).
    """
    NB, BG, NG, R = cfg.NB, cfg.BG, cfg.NG, cfg.R
    block_off = np.zeros((R, 2, NB), dtype=np.int64)
    calls = []  # (g, r, chunk_off, n_chunks)
    off = 0
    for g in range(NG):
        bs = range(g * BG, min((g + 1) * BG, NB))
        for r in range(R):
            cc = 0
            for par in (0, 1):
                for b in bs:
                    block_off[r, par, b] = off + cc
                    cc += int(nch[r, par, b])
            calls.append((g, r, off, cc))
            off += cc
    return block_off, calls, off  # off == total chunks CT


def _prepare_core(cfg, core, nch, block_off, CT, edge_rows, edge_cols, edge_vals):
    """Build this core's linear edge arrays (idx, lrow, val) of length CT*128."""
    R, NPC = cfg.R, cfg.NPC
    lin_idx = np.zeros(CT * P, dtype=np.int16)
    lin_lrow = np.zeros(CT * P, dtype=np.float32)
    lin_val = np.zeros(CT * P, dtype=np.float32)
    for r in range(R):
        er = np.asarray(edge_rows[r])
        m = (er // NPC) == core
        rows = er[m] % NPC
        cols = np.asarray(edge_cols[r])[m]
        vals = np.asarray(edge_vals[r])[m]
        b = rows // P
        lrow = rows % P
        par = cols & 1
        idx = cols >> 1
        grp = par.astype(np.int64) * cfg.NB + b
        order = np.argsort(grp, kind="stable")
        gs = grp[order]
        # rank within each (par, b) bucket
        starts = np.r_[0, np.flatnonzero(np.diff(gs)) + 1]
        sizes = np.diff(np.r_[starts, len(gs)])
        rank = np.arange(len(gs)) - np.repeat(starts, sizes)
        pg, bg_ = gs // cfg.NB, gs % cfg.NB
        pos = block_off[r, pg, bg_] * P + rank
        lin_idx[pos] = idx[order].astype(np.int16)
        lin_lrow[pos] = lrow[order].astype(np.float32)
        lin_val[pos] = vals[order].astype(np.float32)
    return lin_idx, lin_lrow, lin_val


def _wrap_idx(cfg, calls, CT, lin_idx):
    """SWDGE index layout: within each call, edge i -> [i%16, i//16], x8 groups.

    A chunk-aligned sub-range of a call is itself a valid wrap (the column
    offset is 8 per chunk), so gather calls may be split device-side without
    re-wrapping.
    """
    idx_arr = np.zeros((P, CT * 8), dtype=np.int16)
    for (_, _, off, cc) in calls:
        if cc == 0:
            continue
        w = lin_idx[off * P : (off + cc) * P].reshape(cc * 8, 16).T  # [16, cc*8]
        idx_arr[:, off * 8 : (off + cc) * 8] = np.tile(w, (8, 1))
    return idx_arr


def _tag(cg):
    """Per-chunk one-hot build route: 'M' host-DMA, 'V' DVE, 'A' ACT."""
    t = cg % 30
    if t < 20:
        return "M"
    if t < 27:
        return "V"
    return "A"


def _dma_chunks(calls):
    """Chunks whose scatter matrix is host-built, in call-major order."""
    out = []
    for (_, _, off, cc) in calls:
        out.extend(cg for cg in range(off, off + cc) if _tag(cg) == "M")
    return out


def _build(cfg, nch, block_off, calls, CT, skip=()):
    f32 = mybir.dt.float32
    bf16 = mybir.dt.bfloat16
    nc = bacc.Bacc(
        "TRN2",
        target_bir_lowering=False,
        debug=False,
        num_devices=cfg.ncores,
        num_swdge_queues=4,
    )
    D, R, BG, NG, NB, RD1 = cfg.D, cfg.R, cfg.BG, cfg.NG, cfg.NB, cfg.RD1
    xpair = nc.dram_tensor("xpair", [cfg.NSRC, 2 * D], bf16, kind="ExternalInput").ap()
    x_own = nc.dram_tensor("x_own", [NG, P, BG, D], f32, kind="ExternalInput").ap()
    idx_d = nc.dram_tensor("idx", [P, CT * 8], mybir.dt.int16, kind="ExternalInput").ap()
    sc_d = nc.dram_tensor("sc", [P, 4, CT], f32, kind="ExternalInput").ap()
    iota_d = nc.dram_tensor("iota", [P, P], f32, kind="ExternalInput").ap()
    out_d = nc.dram_tensor("out", [cfg.NBP, RD1], f32, kind="ExternalOutput").ap()
    dma_cgs = _dma_chunks(calls)
    TOTM = max(1, len(dma_cgs))
    mv_d = nc.dram_tensor("mvs", [P, TOTM, P], bf16, kind="ExternalInput").ap()
    mslot = {cg: j for j, cg in enumerate(dma_cgs)}

    calls_by_g = {}
    for (g, r, off, cc) in calls:
        calls_by_g.setdefault(g, {})[r] = (off, cc)

    with tile.TileContext(nc) as tc, ExitStack() as ctx:
        cpool = ctx.enter_context(tc.tile_pool(name="c", bufs=1))
        gpool = ctx.enter_context(tc.tile_pool(name="g", bufs=26))
        hpool = ctx.enter_context(tc.tile_pool(name="h", bufs=3))
        ipool = ctx.enter_context(tc.tile_pool(name="i", bufs=3))
        kpool = ctx.enter_context(tc.tile_pool(name="k", bufs=3))
        mpool = ctx.enter_context(tc.tile_pool(name="m", bufs=8))
        opool = ctx.enter_context(tc.tile_pool(name="o", bufs=2))
        ppool = ctx.enter_context(tc.tile_pool(name="p", bufs=4, space="PSUM"))
        qpool = ctx.enter_context(tc.tile_pool(name="q", bufs=1, space="PSUM"))
        spool = ctx.enter_context(tc.tile_pool(name="s", bufs=2, space="PSUM"))

        iota_t = cpool.tile([P, P], f32)
        nc.sync.dma_start(out=iota_t[:], in_=iota_d[:])

        # iota in PSUM: forces DVE tensor_scalar into 1x single-port mode so
        # it never locks the SBUF port pair GPSIMD needs for SWDGE descriptors.
        iota_ps = qpool.tile([P, P], f32, space="PSUM")
        nc.scalar.copy(iota_ps[:], iota_t[:])

        nsub_q = 0
        for g in range(NG):
            bs = list(range(g * BG, min((g + 1) * BG, NB)))
            goff = calls_by_g[g][0][0]
            gend = calls_by_g[g][R - 1][0] + calls_by_g[g][R - 1][1]
            ccg = gend - goff
            # stream this group's gather indices and one-hot scalars
            idx_t = ipool.tile([P, ccg * 8], mybir.dt.int16)
            nc.sync.dma_start(out=idx_t[:], in_=idx_d[:, goff * 8 : gend * 8])
            sct = kpool.tile([P, 4, ccg], f32)
            nc.sync.dma_start(out=sct[:], in_=sc_d[:, :, goff:gend])
            ot = opool.tile([P, BG, RD1], f32)
            if "identity" not in skip:
                # scalar (ACT) HWDGE queue: keeps the sync queue free for the
                # gather-critical idx/sc loads.
                nc.scalar.dma_start(out=ot[:, :, R * D :], in_=x_own[g])
            for r in range(R):
                off, cc = calls_by_g[g][r]
                if cc == 0 or "gather" in skip:
                    continue
                # Split each call into 4 sub-gathers cycling the 4 SWDGE
                # queues: each queue's descriptors are generated by a
                # different Q7 core pair, so descriptor generation (the
                # Q7-software bottleneck) pipelines across pairs.
                subs = []
                nsub = min(4, cc)
                per = (cc + nsub - 1) // nsub
                soff = off
                while soff < off + cc:
                    scc = min(per, off + cc - soff)
                    t = gpool.tile([P, scc, 2 * D], bf16)
                    nc.gpsimd.dma_gather(
                        out_ap=t[:],
                        in_ap=xpair[:],
                        idxs_ap=idx_t[:, (soff - goff) * 8 : (soff - goff + scc) * 8],
                        num_idxs=scc * P,
                        num_idxs_reg=scc * P,
                        elem_size=2 * D,
                        single_packet=False,
                        queue_num=nsub_q % 4,
                    )
                    nsub_q += 1
                    subs.append((soff, scc, t))
                    soff += scc
                # host-built scatter matrices for this call's 'M' chunks
                mtiles = [cg for cg in range(off, off + cc) if _tag(cg) == "M"]
                if mtiles and "mvdma" not in skip:
                    mvs = hpool.tile([P, len(mtiles), P], bf16)
                    j0 = mslot[mtiles[0]]
                    nc.scalar.dma_start(
                        out=mvs[:], in_=mv_d[:, j0 : j0 + len(mtiles), :]
                    )
                for b4, b in enumerate(bs):
                    if "mm" in skip:
                        continue
                    total = int(nch[r, 0, b] + nch[r, 1, b])
                    acc = ppool.tile([P, D], f32, space="PSUM")
                    k = 0
                    for par in (0, 1):
                        n = int(nch[r, par, b])
                        if n == 0:
                            continue
                        boff = int(block_off[r, par, b])
                        for ci in range(n):
                            cg = boff + ci
                            cgl = cg - goff
                            sub = subs[min((cg - off) // per, len(subs) - 1)]
                            t = sub[2]
                            cl = cg - sub[0]
                            tag = _tag(cg)
                            if tag == "M" and "mvdma" not in skip:
                                mv_ap = mvs[:, mslot[cg] - j0, :]
                            elif tag == "A" and "act" not in skip:
                                # ACT route: val * relu(1 - (iota - lrow)^2)
                                mv = mpool.tile([P, P], bf16)
                                sq = spool.tile([P, P], f32, space="PSUM")
                                nc.scalar.activation(
                                    sq[:],
                                    iota_ps[:],
                                    mybir.ActivationFunctionType.Square,
                                    bias=sct[:, 2, cgl : cgl + 1],
                                )
                                nc.scalar.activation(
                                    mv[:],
                                    sq[:],
                                    mybir.ActivationFunctionType.Relu,
                                    bias=sct[:, 1, cgl : cgl + 1],
                                    scale=sct[:, 3, cgl : cgl + 1],
                                )
                                mv_ap = mv[:]
                            else:
                                mv = mpool.tile([P, P], bf16)
                                nc.vector.tensor_scalar(
                                    out=mv[:],
                                    in0=iota_ps[:],
                                    scalar1=sct[:, 0, cgl : cgl + 1],
                                    scalar2=sct[:, 1, cgl : cgl + 1],
                                    op0=mybir.AluOpType.is_equal,
                                    op1=mybir.AluOpType.mult,
                                )
                                mv_ap = mv[:]
                            if "matmul" not in skip:
                                nc.tensor.matmul(
                                    out=acc[:],
                                    lhsT=mv_ap,
                                    rhs=t[:, cl, par * D : (par + 1) * D],
                                    start=(k == 0),
                                    stop=(k == total - 1),
                                )
                            k += 1
                    if "matmul" not in skip and "copy" not in skip:
                        nc.scalar.copy(ot[:, b4, r * D : (r + 1) * D], acc[:])
            for b4, b in enumerate(bs):
                if "out" in skip:
                    continue
                nc.scalar.dma_start(
                    out=out_d[b * P : (b + 1) * P, :], in_=ot[:, b4, :]
                )
    nc.compile()
    return nc


_CACHE = {}


def _get_kernel(cfg, nch, block_off, calls, CT):
    key = (cfg.N, cfg.D, cfg.R, cfg.ncores, cfg.ACT_EVERY, nch.tobytes())
    if key not in _CACHE:
        _CACHE[key] = _build(cfg, nch, block_off, calls, CT)
    return _CACHE[key]


def run(x, edge_rows, edge_cols, edge_vals, cfg=None, trace=False, tmpdir=None):
    x = np.ascontiguousarray(np.asarray(x, dtype=np.float32))
    edge_rows = np.asarray(edge_rows, dtype=np.int64)
    edge_cols = np.asarray(edge_cols, dtype=np.int64)
    edge_vals = np.asarray(edge_vals, dtype=np.float32)
    if cfg is None:
        cfg = Config(x.shape[0], x.shape[1], edge_rows.shape[0])

    nch = _schedule(cfg, edge_rows, edge_cols)
    block_off, calls, CT = _layout(cfg, nch)
    nc = _get_kernel(cfg, nch, block_off, calls, CT)

    iota = np.broadcast_to(np.arange(P, dtype=np.float32), (P, P))
    xpair = np.ascontiguousarray(x.reshape(cfg.NSRC, 2 * cfg.D)).astype(BF16)
    dma_cgs = np.asarray(_dma_chunks(calls), dtype=np.int64)
    TOTM = max(1, len(dma_cgs))
    in_maps = []
    for core in range(cfg.ncores):
        lin_idx, lin_lrow, lin_val = _prepare_core(
            cfg, core, nch, block_off, CT, edge_rows, edge_cols, edge_vals
        )
        idx_arr = _wrap_idx(cfg, calls, CT, lin_idx)
        lrow_arr = np.ascontiguousarray(lin_lrow.reshape(CT, P).T)
        val_arr = np.ascontiguousarray(lin_val.reshape(CT, P).T)
        sc = np.ascontiguousarray(
            np.stack([lrow_arr, val_arr, -lrow_arr, -val_arr], axis=1)
        )
        # host-built scatter matrices for the 'M'-tagged chunks
        mvs_all = np.zeros((TOTM, P, P), dtype=np.float32)
        if len(dma_cgs):
            lr = lin_lrow.reshape(CT, P)[dma_cgs].astype(np.int64)  # [TOTM, P]
            vv = lin_val.reshape(CT, P)[dma_cgs]
            mvs_all[
                np.arange(len(dma_cgs))[:, None], np.arange(P)[None, :], lr
            ] = vv
        mvs_all = np.ascontiguousarray(mvs_all.transpose(1, 0, 2)).astype(BF16)
        xpad = np.zeros((cfg.NG * cfg.BG * P, cfg.D), dtype=np.float32)
        xpad[: cfg.NPC] = x[core * cfg.NPC : (core + 1) * cfg.NPC]
        x_own = np.ascontiguousarray(
            xpad.reshape(cfg.NG, cfg.BG, P, cfg.D).transpose(0, 2, 1, 3)
        )
        in_maps.append(
            {
                "xpair": xpair,
                "x_own": x_own,
                "idx": idx_arr,
                "sc": sc,
                "iota": np.ascontiguousarray(iota),
                "mvs": mvs_all,
            }
        )

    res = run_bass_kernel_spmd(
        nc, in_maps, list(range(cfg.ncores)), trace=trace, tmpdir=tmpdir
    )
    out = np.concatenate(
        [res.results[i]["out"][: cfg.NPC] for i in range(cfg.ncores)], axis=0
    )
    return out, res


def kernel(x, edge_rows, edge_cols, edge_vals):
    out, _ = run(x, edge_rows, edge_cols, edge_vals)
    return out
